# revision 4
# baseline (speedup 1.0000x reference)
"""Trainium2 Bass kernel v2 for GNN multi-head cross-attention message passing.

Math (see reference): per edge e: score[e,h,g] = qh[A[e],h,:] . kh[B[e],g,:]
segment-MEAN over destination A -> softmax over g -> att @ vh -> Wc projection.

Algebra (same as v1): within a segment qh[n] is constant so
    sums[n,h,g] = qh[n,h,:] . S[n,g,:],  S = (segment-sum of raw k rows) @ Wk^T
so the [E,H,H] per-edge tensor is never materialized and the k-projection
happens after aggregation.

v2 speedups over v1:
- fp8 (e4m3) edge stream, packed partition-major so DMA runs full rate
  (contiguous chunks >= 512B); halves the dominant HBM traffic.
- scatter (segment-sum) via fp8 DoubleRow matmuls: two 128-edge tiles are
  accumulated per PE instruction.  Most edges land in "dense" tiles whose
  scatter matrix is the identity (host places each node's first DN edges at
  partition = node), so no one-hot needs to be built for them; only overflow
  edges use dynamic one-hot tiles (is_equal against an iota).
- q/Wq and U/Wk projections as single fp8 DoubleRow matmuls (256-deep
  contraction in one instruction).  Weights are pre-scaled x16 into fp8
  normal range; the 1/256 compensation is folded into the softmax 1/cnt
  scale.  v/Wv and ov/Wc stay bf16 (output precision).
- score & V phases: broadcast-multiply + log-tree adds in bf16 (2x DVE
  mode), balanced across DVE and GpSimd; PSUM->SBUF copies spread across
  ACT/GpSimd; final projection is DMA'd to HBM straight from PSUM.
"""

import numpy as np
import ml_dtypes

import concourse.bass as bass
import concourse.mybir as mybir
import concourse.tile as tile
from concourse.bass_utils import run_bass_kernel_spmd
from concourse.masks import make_identity

# ---------------------------------------------------------------- constants
NCORES = 8
N_NODES = 50000
EMB = 256
H = 8
D = 32
P = 128

NPC = N_NODES // NCORES          # 6250 nodes per core
NB = (NPC + P - 1) // P          # 49 blocks of 128 nodes per core
NPC_PAD = NB * P                 # 6272

DENSE = 16                       # dense (identity-scatter) tiles per block
WSCALE = 16.0                    # fp8 pre-scale for Wq / Wk
CHUNK = 4                        # node blocks per ke DMA chunk

FP = mybir.dt.float32
BF = mybir.dt.bfloat16
F8 = mybir.dt.float8e4
DR = mybir.MatmulPerfMode.DoubleRow

NPF8 = ml_dtypes.float8_e4m3
NPBF = ml_dtypes.bfloat16


# ------------------------------------------------------- sync-wait splitting
# The staged walrus accepts only ONE sync-wait command per instruction.
# Post-pass: hoist all but one wait of each over-limit instruction onto
# same-engine Drain carriers placed immediately before it.
_WS_COUNTER = [0]


def _split_sync_waits(nc, maxw=1):
    for f in nc.m.functions:
        for blk in f.blocks:
            insts = blk.instructions
            out = []
            changed = False
            for ins in insts:
                si = ins.sync_info
                if si is not None and len(si.on_wait) > maxw:
                    waits = list(si.on_wait)
                    k = len(waits) - maxw
                    for i in range(0, k, maxw):
                        _WS_COUNTER[0] += 1
                        d = mybir.InstDrain(
                            name=f"I-wsplit-{_WS_COUNTER[0]}", ins=[], outs=[]
                        )
                        d.engine = ins.engine
                        d.sync_info = mybir.SyncInfo(
                            on_wait=waits[i : i + maxw], on_update=[]
                        )
                        out.append(d)
                    si.on_wait = waits[k:]
                    changed = True
                out.append(ins)
            if changed:
                blk.instructions = out


# ------------------------------------------------------------- device kernel
def build_nc(dyn_tiles, split_waits=True):
    """dyn_tiles[b] = dynamic one-hot tiles in block b (even, shared across
    cores).  Total tiles per block = DENSE + dyn_tiles[b]."""
    tiles = [DENSE + int(d) for d in dyn_tiles]
    ET = int(sum(tiles))
    ETD = int(sum(dyn_tiles))
    DYNMAX = int(max(dyn_tiles)) if ETD else 0
    t0 = np.concatenate([[0], np.cumsum(tiles)]).astype(int)      # global tile idx
    d0 = np.concatenate([[0], np.cumsum(dyn_tiles)]).astype(int)  # global dyn idx

    nc = bass.Bass("TRN2", target_bir_lowering=False, debug=False,
                   num_devices=NCORES)

    # per-core inputs
    qp_d = nc.dram_tensor("qp", [P, 2 * NPC_PAD], F8, kind="ExternalInput")
    vp_d = nc.dram_tensor("vp", [P, 2 * NPC_PAD], BF, kind="ExternalInput")
    ke_d = nc.dram_tensor("ke", [P, ET * EMB], F8, kind="ExternalInput")
    wqp_d = nc.dram_tensor("wqp", [P, 2 * EMB], F8, kind="ExternalInput")
    wkp_d = nc.dram_tensor("wkp", [P, 2 * EMB], F8, kind="ExternalInput")
    wvp_d = nc.dram_tensor("wvp", [P, 2 * EMB], BF, kind="ExternalInput")
    wcp_d = nc.dram_tensor("wcp", [P, 2 * EMB], BF, kind="ExternalInput")
    invc_d = nc.dram_tensor("invc", [P, NB], FP, kind="ExternalInput")
    if ETD:
        aloc_d = nc.dram_tensor("aloc", [P, ETD], BF, kind="ExternalInput")

    out_d = nc.dram_tensor("out_shard", [NPC_PAD, EMB], FP, kind="ExternalOutput")

    with tile.TileContext(nc) as tc:
        with (
            tc.tile_pool(name="const", bufs=1) as cp,
            tc.tile_pool(name="work", bufs=6) as wp,
            tc.tile_pool(name="big", bufs=3) as bp,
            tc.tile_pool(name="kep", bufs=3) as kp,
            tc.tile_pool(name="psA", bufs=2, space="PSUM") as pA,
            tc.tile_pool(name="psB", bufs=2, space="PSUM") as pB,
            tc.tile_pool(name="psT", bufs=2, space="PSUM") as pT,
        ):
            # ---------------- constants
            iota_i = cp.tile([P, P], mybir.dt.int32)
            nc.gpsimd.iota(iota_i[:], pattern=[[1, P]], base=0, channel_multiplier=0)
            iota_b = cp.tile([P, P], BF)
            nc.vector.tensor_copy(iota_b[:], iota_i[:])
            ident = cp.tile([P, P], FP)
            make_identity(nc, ident[:])
            ident_b = cp.tile([P, P], BF)
            nc.vector.tensor_copy(ident_b[:], ident[:])
            ident_8 = cp.tile([P, P], F8)
            nc.vector.tensor_copy(ident_8[:], ident[:])
            idp = cp.tile([P, 2, P], F8)       # identity pair for dense scatter
            nc.vector.tensor_copy(idp[:, 0, :], ident[:])
            nc.vector.tensor_copy(idp[:, 1, :], ident[:])

            qsb = cp.tile([P, 2 * NPC_PAD], F8)
            nc.scalar.dma_start(qsb[:], qp_d[:])
            vsb = cp.tile([P, 2 * NPC_PAD], BF)
            nc.scalar.dma_start(vsb[:], vp_d[:])
            wqp = cp.tile([P, 2, EMB], F8)
            nc.scalar.dma_start(wqp[:], wqp_d[:].rearrange("p (i c) -> p i c", i=2))
            wkp = cp.tile([P, 2, EMB], F8)
            nc.scalar.dma_start(wkp[:], wkp_d[:].rearrange("p (i c) -> p i c", i=2))
            wvp = cp.tile([P, 2, EMB], BF)
            nc.scalar.dma_start(wvp[:], wvp_d[:].rearrange("p (i c) -> p i c", i=2))
            wcp = cp.tile([P, 2, EMB], BF)
            nc.scalar.dma_start(wcp[:], wcp_d[:].rearrange("p (i c) -> p i c", i=2))
            invc_sb = cp.tile([P, NB], FP)
            nc.scalar.dma_start(invc_sb[:], invc_d[:])
            if ETD:
                aloc_sb = cp.tile([P, ETD], BF)
                nc.scalar.dma_start(aloc_sb[:], aloc_d[:])


            # -------- software-pipelined main loop (6 stages, lag 1 each).
            # Engine streams execute in order, so every cross-engine
            # dependency must point at least one iteration back (or to an
            # earlier-emitted op of the same iteration).  Stages per
            # iteration i (block index in brackets):
            #   SE [i-3]: exp                       (ACT, emitted first)
            #   S0a[i]  : qv proj, one-hot, scatter, U copy, U transpose
            #   S0b[i-1]: uT copy (gp), S proj (PE), S copy (ACT)
            #   S1 [i-2]: score mul + tree          (DVE)
            #   SA [i-3]: den, recip, att           (DVE)
            #   SV [i-4]: V mul + tree              (gp)
            #   S2 [i-5]: ov transpose, Wc proj, out DMA
            qv = qsb[:].rearrange("p (i n) -> p i n", i=2)
            vv = vsb[:].rearrange("p (i n) -> p i n", i=2)

            chunk_starts = set(range(0, NB, CHUNK))
            ke_tiles = {}
            S = {}   # S[b] = dict of live tiles for block b

            def stage_e(b):
                st = S[b]
                ex = wp.tile([P, H * H], BF, tag="ex")
                with nc.allow_low_precision(reason="softmax bf16"):
                    nc.scalar.activation(
                        out=ex[:],
                        in_=st.pop("sc")[:].rearrange("p h g o -> p (h g o)"),
                        func=mybir.ActivationFunctionType.Exp,
                        scale=invc_sb[:, b:b + 1])
                st["ex"] = ex

            def issue_chunk(b):
                c1 = min(b + CHUNK, NB)
                ket = kp.tile([P, int(t0[c1] - t0[b]), EMB], F8, tag="ke")
                nc.sync.dma_start(
                    ket[:],
                    ke_d[:, t0[b] * EMB:t0[c1] * EMB]
                    .rearrange("p (t c) -> p t c", c=EMB),
                )
                for bb in range(b, c1):
                    ke_tiles[bb] = (ket, int(t0[bb] - t0[b]))

            def stage0a(b):
                ke, lt = ke_tiles.pop(b)
                ns = slice(b * P, (b + 1) * P)
                dyn = int(dyn_tiles[b])

                ps_qv = pA.tile([P, 2 * EMB], FP, space="PSUM", tag="qv")
                nc.tensor.matmul(out=ps_qv[:, 0:EMB], lhsT=qv[:, :, ns],
                                 rhs=wqp[:], start=True, stop=True,
                                 perf_mode=DR)
                nc.tensor.matmul(out=ps_qv[:, EMB:2 * EMB], lhsT=vv[:, 0, ns],
                                 rhs=wvp[:, 0, :], start=True, stop=False)
                nc.tensor.matmul(out=ps_qv[:, EMB:2 * EMB], lhsT=vv[:, 1, ns],
                                 rhs=wvp[:, 1, :], start=False, stop=True)
                qv_sb = wp.tile([P, 2 * EMB], BF, tag="qv_sb")
                nc.scalar.copy(qv_sb[:], ps_qv[:])

                oh = None
                if dyn:
                    oh = wp.tile([P, DYNMAX, P], F8, tag="oh")
                    nc.vector.tensor_tensor(
                        out=oh[:, 0:dyn, :],
                        in0=iota_b[:].unsqueeze(1).to_broadcast([P, dyn, P]),
                        in1=aloc_sb[:, d0[b]:d0[b] + dyn].unsqueeze(2)
                            .to_broadcast([P, dyn, P]),
                        op=mybir.AluOpType.is_equal,
                    )

                ps_u = pB.tile([P, EMB], FP, space="PSUM", tag="U")
                npair = (DENSE + dyn) // 2
                for j in range(npair):
                    tt = 2 * j
                    rhs = ke[:, lt + tt:lt + tt + 2, :]
                    lhs = idp[:] if tt < DENSE else oh[:, tt - DENSE:tt - DENSE + 2, :]
                    nc.tensor.matmul(out=ps_u[:], lhsT=lhs, rhs=rhs,
                                     start=(j == 0), stop=(j == npair - 1),
                                     perf_mode=DR)
                u_sb = wp.tile([P, EMB], BF, tag="u_sb")
                nc.scalar.copy(u_sb[:], ps_u[:])

                tpu = pT.tile([P, 2, P], BF, space="PSUM", tag="tp2")
                for i in range(2):
                    nc.tensor.transpose(tpu[:, i, :], u_sb[:, i * P:(i + 1) * P],
                                        ident_b[:])
                S[b] = {"qv_sb": qv_sb, "tpu": tpu}

            def stage0b(b):
                st = S[b]
                uT2 = wp.tile([P, 2, P], F8, tag="uT2")
                nc.scalar.copy(uT2[:], st.pop("tpu")[:])
                ps_s = pB.tile([P, EMB], FP, space="PSUM", tag="acc")
                nc.tensor.matmul(out=ps_s[:], lhsT=uT2[:], rhs=wkp[:],
                                 start=True, stop=True, perf_mode=DR)
                s_sb = wp.tile([P, EMB], BF, tag="s_sb")
                nc.scalar.copy(s_sb[:], ps_s[:])
                st["s_sb"] = s_sb

            def stage1(b):
                st = S[b]
                qh_sb = st["qv_sb"][:, 0:EMB]
                prod = bp.tile([P, H, H, D], BF, tag="prod")
                nc.vector.tensor_tensor(
                    out=prod[:],
                    in0=qh_sb.rearrange("p (h d) -> p h d", h=H)
                        .unsqueeze(2).to_broadcast([P, H, H, D]),
                    in1=st.pop("s_sb")[:].rearrange("p (g d) -> p g d", g=H)
                        .unsqueeze(1).to_broadcast([P, H, H, D]),
                    op=mybir.AluOpType.mult,
                )
                cur = prod
                w = D
                while w > 2:
                    nxt = bp.tile([P, H, H, w // 2], BF, tag=f"ts{w}")
                    cv = cur[:]
                    with nc.allow_low_precision(reason="bf16 tree"):
                        nc.vector.tensor_tensor(
                            out=nxt[:], in0=cv[:, :, :, 0:w // 2],
                            in1=cv[:, :, :, w // 2:w],
                            op=mybir.AluOpType.add)
                    cur = nxt
                    w //= 2
                sc = wp.tile([P, H, H, 1], FP, tag="sc")
                nc.vector.tensor_tensor(
                    out=sc[:], in0=cur[:][:, :, :, 0:1], in1=cur[:][:, :, :, 1:2],
                    op=mybir.AluOpType.add)
                st["sc"] = sc

            def stage_a(b):
                st = S[b]
                ex = st.pop("ex")
                den = wp.tile([P, H], FP, tag="den")
                nc.vector.tensor_reduce(
                    out=den[:], in_=ex[:].rearrange("p (h g) -> p h g", h=H),
                    axis=mybir.AxisListType.X, op=mybir.AluOpType.add)
                rden = wp.tile([P, H], BF, tag="rden")
                with nc.allow_low_precision(reason="softmax bf16"):
                    nc.vector.reciprocal(rden[:], den[:])
                att = wp.tile([P, H, H], BF, tag="att")
                nc.vector.tensor_tensor(
                    out=att[:],
                    in0=ex[:].rearrange("p (h g) -> p h g", h=H),
                    in1=rden[:].unsqueeze(2).to_broadcast([P, H, H]),
                    op=mybir.AluOpType.mult,
                )
                st["att"] = att

            def stage_v(b):
                st = S[b]
                vh_sb = st.pop("qv_sb")[:, EMB:2 * EMB]
                p2 = bp.tile([P, H, H, D], BF, tag="p2")
                nc.gpsimd.tensor_tensor(
                    out=p2[:],
                    in0=st.pop("att")[:].unsqueeze(3).to_broadcast([P, H, H, D]),
                    in1=vh_sb.rearrange("p (g d) -> p g d", g=H)
                        .unsqueeze(1).to_broadcast([P, H, H, D]),
                    op=mybir.AluOpType.mult,
                )
                cur = p2
                w = H
                while w > 2:
                    nxt = bp.tile([P, H, w // 2, D], BF, tag=f"tv{w}")
                    cv = cur[:]
                    with nc.allow_low_precision(reason="bf16 tree"):
                        nc.gpsimd.tensor_tensor(
                            out=nxt[:], in0=cv[:, :, 0:w // 2, :],
                            in1=cv[:, :, w // 2:w, :],
                            op=mybir.AluOpType.add)
                    cur = nxt
                    w //= 2
                ov = wp.tile([P, H, 1, D], BF, tag="ov")
                with nc.allow_low_precision(reason="bf16 tree"):
                    nc.vector.tensor_tensor(
                        out=ov[:], in0=cur[:][:, :, 0:1, :],
                        in1=cur[:][:, :, 1:2, :],
                        op=mybir.AluOpType.add)
                st["ov"] = ov

            def stage2(b):
                st = S.pop(b)
                ov_f = st["ov"][:].rearrange("p h o d -> p (h o d)")
                ovT2 = wp.tile([P, 2, P], BF, tag="ovT2")
                tpv = pT.tile([P, 2, P], BF, space="PSUM", tag="tp2")
                for i in range(2):
                    nc.tensor.transpose(tpv[:, i, :], ov_f[:, i * P:(i + 1) * P],
                                        ident_b[:])
                nc.scalar.copy(ovT2[:], tpv[:])
                ps_f = pB.tile([P, EMB], FP, space="PSUM", tag="acc")
                nc.tensor.matmul(out=ps_f[:], lhsT=ovT2[:, 0, :],
                                 rhs=wcp[:, 0, :], start=True, stop=False)
                nc.tensor.matmul(out=ps_f[:], lhsT=ovT2[:, 1, :],
                                 rhs=wcp[:, 1, :], start=False, stop=True)
                fin = wp.tile([P, EMB], FP, tag="fin")
                nc.scalar.copy(fin[:], ps_f[:])
                nc.scalar.dma_start(out_d[b * P:(b + 1) * P, :], fin[:])

            def emit(i):
                def ok(b):
                    return 0 <= b < NB
                if i == 0:
                    issue_chunk(0)
                    if CHUNK < NB:
                        issue_chunk(CHUNK)
                if i + 2 * CHUNK in chunk_starts and i + 2 * CHUNK < NB:
                    issue_chunk(i + 2 * CHUNK)
                if ok(i - 3):
                    stage_e(i - 3)
                if ok(i):
                    stage0a(i)
                if ok(i - 1):
                    stage0b(i - 1)
                if ok(i - 2):
                    stage1(i - 2)
                if ok(i - 3):
                    stage_a(i - 3)
                if ok(i - 4):
                    stage_v(i - 4)
                if ok(i - 5):
                    stage2(i - 5)

            for i in range(NB + 5):
                emit(i)

    if split_waits:
        _split_sync_waits(nc)
    return nc


# --------------------------------------------------------------- host prep
def _prep(q, k, v, edge_index, Wq, bq, Wk, bk, Wv, bv, Wc, bc):
    A = np.asarray(edge_index[0], dtype=np.int64)
    B = np.asarray(edge_index[1], dtype=np.int64)
    order = np.argsort(A, kind="stable")
    A_s = A[order]
    B_s = B[order]

    core_lo = np.searchsorted(A_s, np.arange(NCORES) * NPC, side="left")
    core_hi = np.searchsorted(A_s, (np.arange(NCORES) + 1) * NPC, side="left")

    # pass 1: per-core overflow counts per block
    per_core = []
    ovf_counts = np.zeros((NCORES, NB), dtype=np.int64)
    for o in range(NCORES):
        a = A_s[core_lo[o]:core_hi[o]] - o * NPC
        bsrc = B_s[core_lo[o]:core_hi[o]]
        first = np.searchsorted(a, a, side="left")
        r = np.arange(len(a)) - first                  # rank within dest node
        blk = a // P
        ovf = r >= DENSE
        ovf_counts[o] = np.bincount(blk[ovf], minlength=NB)
        per_core.append((a, bsrc, r, blk, ovf))

    dyn_tiles = ((ovf_counts.max(axis=0) + P - 1) // P)
    dyn_tiles = (dyn_tiles + (dyn_tiles % 2)).astype(int)          # even
    tiles = DENSE + dyn_tiles
    ET = int(tiles.sum())
    ETD = int(dyn_tiles.sum())
    t0 = np.concatenate([[0], np.cumsum(tiles)]).astype(np.int64)
    d0 = np.concatenate([[0], np.cumsum(dyn_tiles)]).astype(np.int64)

    k8 = np.asarray(k, np.float32).astype(NPF8)
    kes, alocs = [], []
    for o in range(NCORES):
        a, bsrc, r, blk, ovf = per_core[o]
        ke3 = np.zeros((P, ET, EMB), dtype=NPF8)
        # dense slots: tile t0[blk]+r, partition a%P
        dm = ~ovf
        ke3[a[dm] % P, t0[blk[dm]] + r[dm]] = k8[bsrc[dm]]
        # overflow slots: packed sequentially per block
        if ETD:
            al = np.full((P, ETD), -1.0, dtype=np.float32)
            ob = blk[ovf]
            start = np.searchsorted(ob, np.arange(NB))
            pos = np.arange(len(ob)) - start[ob]
            ke3[pos % P, t0[ob] + DENSE + pos // P] = k8[bsrc[ovf]]
            al[pos % P, d0[ob] + pos // P] = (a[ovf] % P).astype(np.float32)
            alocs.append(al.astype(NPBF))
        kes.append(np.ascontiguousarray(ke3.reshape(P, ET * EMB)))

    cnt_nodes = np.bincount(A, minlength=N_NODES).astype(np.float32)
    inv_full = 1.0 / np.maximum(cnt_nodes, 1.0) / (WSCALE * WSCALE)
    invcs = []
    for o in range(NCORES):
        s = np.ones(NPC_PAD, dtype=np.float32)
        s[:NPC] = inv_full[o * NPC:(o + 1) * NPC]
        invcs.append(np.ascontiguousarray(s.reshape(NB, P).T))

    q = np.asarray(q, np.float32)
    v = np.asarray(v, np.float32)
    qps, vps = [], []
    for o in range(NCORES):
        qp = np.zeros((P, 2, NPC_PAD), dtype=NPF8)
        vp = np.zeros((P, 2, NPC_PAD), dtype=NPBF)
        qs = q[o * NPC:(o + 1) * NPC]
        vs = v[o * NPC:(o + 1) * NPC]
        qp[:, 0, :NPC] = qs[:, 0:P].T.astype(NPF8)
        qp[:, 1, :NPC] = qs[:, P:EMB].T.astype(NPF8)
        vp[:, 0, :NPC] = vs[:, 0:P].T.astype(NPBF)
        vp[:, 1, :NPC] = vs[:, P:EMB].T.astype(NPBF)
        qps.append(qp.reshape(P, 2 * NPC_PAD))
        vps.append(vp.reshape(P, 2 * NPC_PAD))

    def pair_w(W, dt, scale=1.0):
        # wp[c, i, oc] = scale * W[oc, 128*i + c]
        Wt = np.asarray(W, np.float32).T * scale       # [c, oc]
        out = np.stack([Wt[0:P, :], Wt[P:EMB, :]], axis=1)  # [128, 2, 256]
        return np.ascontiguousarray(out.reshape(P, 2 * EMB).astype(dt))

    com = {
        "wqp": pair_w(Wq, NPF8, WSCALE),
        "wkp": pair_w(Wk, NPF8, WSCALE),
        "wvp": pair_w(Wv, NPBF),
        "wcp": pair_w(Wc, NPBF),
    }
    in_maps = []
    for o in range(NCORES):
        m = dict(com)
        m["qp"] = qps[o]
        m["vp"] = vps[o]
        m["ke"] = kes[o]
        m["invc"] = invcs[o]
        if ETD:
            m["aloc"] = alocs[o]
        in_maps.append(m)
    return dyn_tiles.tolist(), in_maps


_LAST = {}


def kernel(q, k, v, edge_index, Wq, bq, Wk, bk, Wv, bv, Wc, bc, latent=None,
           _want_results=False, _trace=False):
    bq = np.asarray(bq, np.float32)
    bk = np.asarray(bk, np.float32)
    bv = np.asarray(bv, np.float32)
    bc = np.asarray(bc, np.float32)
    # Non-zero biases are folded host-side, exactly:
    #   (x + b@inv(W.T)) @ W.T == x@W.T + b
    # For bk this also reproduces the cnt-scaled bias, since adding a
    # constant row to every k row adds cnt[n] copies of it to each segment
    # sum.  bc is added to the final output directly.
    if np.any(bq != 0):
        q = np.asarray(q, np.float32) + np.linalg.solve(
            np.asarray(Wq, np.float32).T.astype(np.float64),
            bq.astype(np.float64)).astype(np.float32)
    if np.any(bv != 0):
        v = np.asarray(v, np.float32) + np.linalg.solve(
            np.asarray(Wv, np.float32).T.astype(np.float64),
            bv.astype(np.float64)).astype(np.float32)
    if np.any(bk != 0):
        k = np.asarray(k, np.float32) + np.linalg.solve(
            np.asarray(Wk, np.float32).T.astype(np.float64),
            bk.astype(np.float64)).astype(np.float32)
    dyn_tiles, in_maps = _prep(q, k, v, edge_index,
                               Wq, bq, Wk, bk, Wv, bv, Wc, bc)
    key = tuple(dyn_tiles)
    if _LAST.get("key") != key:
        _LAST["nc"] = build_nc(dyn_tiles)
        _LAST["key"] = key
    nc = _LAST["nc"]

    res = run_bass_kernel_spmd(nc, in_maps, core_ids=list(range(NCORES)),
                               trace=_trace)
    out = np.empty((N_NODES, EMB), dtype=np.float32)
    for o in range(NCORES):
        out[o * NPC:(o + 1) * NPC] = res.results[o]["out_shard"][:NPC]
    if np.any(bc != 0):
        out += bc.reshape(1, EMB)
    if _want_results:
        return out, res
    return out
